# revision 19
# baseline (speedup 1.0000x reference)
"""Trainium2 Bass kernel for nn_Attention_layer (per-label MLP attention).

Computes, for full inputs:
    h = relu(cat(label_emb, unlabel_emb) @ W1 + b1)        [N, B, H]
    scores = h @ W2 + b2                                   [N, B]
    out = softmax(scores.T * dis_lab, axis=1)              [B, N]

Distribution: pure data-parallel over batch B across 8 NeuronCores
(B=1024 -> 128 rows/core). No collectives; softmax is over the station
axis N which stays local to a core.

Host prep: W2 is folded into W1 (W1' = W1 * w2 per column), columns
sorted so positive-w2 columns come first; then
    scores = sum_pos relu(h') + sum_neg min(h', 0)
Activations/weights are cast to bf16 and label/unlabel embeddings are
pre-transposed to [K, batch] layout so all device DMAs are contiguous.

Device (per core, per station n):
  PE:  psum[128b, 1024] = I @ unl_h' (start) + xlabT_k @ W1'_k   (bf16)
       (the identity matmul injects the shared unlabel contribution -
        an exact rank-128 K-extension - cheaper than any vector add)
  relu+signed-sum per station, balanced across engines, 3 modes:
    1: ACT activation(Relu, accum_out) on [:jpos] / scale=-1 on [jpos:]
    2: ACT relu psum->sbuf bf16, then DVE reduce_sum x2
    3: DVE tensor_scalar(max/min 0, op1=add, accum_out) x2
  Tail: scores * dis_lab, stable softmax over the 64 stations.
"""

import os
import sys

for _p in (
    "/root/.axon_site",
    "/root/.axon_site/_ro/trn_rl_repo",
    "/root/.axon_site/_ro/pypackages",
):
    if _p not in sys.path and os.path.isdir(_p):
        sys.path.append(_p)

import ml_dtypes
import numpy as np

import concourse.bass as bass
import concourse.mybir as mybir
import concourse.tile as tile
from concourse import bacc
from concourse.bass_utils import run_bass_kernel_spmd
from concourse.masks import make_identity

N, B, EMB, UEMB, H = 64, 1024, 256, 256, 1024
N_CORES = 8
BS = B // N_CORES  # 128 batch rows per core
KL = EMB // 128  # label-emb contraction chunks
KU = UEMB // 128  # unlabel-emb contraction chunks
F32 = mybir.dt.float32
BF16 = mybir.dt.bfloat16

# Tuning knobs.
_RELU_MODE_PAT = [1, 2, 2, 3, 1, 2, 2, 3]
RELU_MODE = lambda n: (1 if n == N - 2 else 3) if n >= N - 2 else _RELU_MODE_PAT[n % len(_RELU_MODE_PAT)]  # noqa: E731
ADD_ON_DVE = lambda n0: False  # noqa: E731

PROFILE = False
LAST_EXEC_NS = None
TRACE_DIR = None

_cache = {}


def _build(jpos, b2val, zero_b1):
    nc = bacc.Bacc("TRN2", target_bir_lowering=False, debug=False,
                   num_devices=N_CORES)
    xlabT = nc.dram_tensor("xlabT", [KL, 128, N, BS], BF16,
                           kind="ExternalInput").ap()
    xunlT = nc.dram_tensor("xunlT", [KU, 128, BS], BF16,
                           kind="ExternalInput").ap()
    dis = nc.dram_tensor("dis", [BS, N], F32, kind="ExternalInput").ap()
    w1p = nc.dram_tensor("w1p", [2, 128, 2 * H], BF16,
                         kind="ExternalInput").ap()
    b1p = nc.dram_tensor("b1p", [H], F32, kind="ExternalInput").ap()
    out = nc.dram_tensor("out", [BS, N], F32, kind="ExternalOutput").ap()

    with tile.TileContext(nc) as tc:
        _emit(tc, out, xlabT, xunlT, dis, w1p, b1p, jpos, b2val, zero_b1)
    nc.compile()
    return nc


def _emit(tc, out, xlabT_d, xunlT_d, dis, w1p, b1p, jpos, b2val, zero_b1):
    nc = tc.nc
    AF = mybir.ActivationFunctionType
    ALU = mybir.AluOpType

    with tc.tile_pool(name="consts", bufs=1) as consts:
        # --- constants / weights ---
        ident = consts.tile([128, 128], BF16, tag="ident")
        make_identity(nc, ident)

        w1pair = []
        for pg in range(2):
            t = consts.tile([128, 2, H], BF16, tag=f"w1pg_{pg}")
            w1pair.append(t)
        w1sb = [w1pair[0][:, 0, :], w1pair[0][:, 1, :],
                w1pair[1][:, 0, :], w1pair[1][:, 1, :]]
        # DMA schedule: data arrives in the order the PE consumes it.
        # ACT queue: unl weights (PE-queue head), then tail-only tensors.
        # SP queue: w1[0], first label chunk, w1[1], then remaining chunks.
        # GPSIMD SWDGE: tiny unlabel embeddings (dispatches earliest).
        xunlT = []
        for k in range(KU):
            t = consts.tile([128, 128], BF16, tag=f"xunlT_{k}")
            nc.gpsimd.dma_start(out=t, in_=xunlT_d[k])
            xunlT.append(t)
        nc.scalar.dma_start(out=w1pair[1], in_=w1p[1])

        xlabT = consts.tile([128, KL, N, 128], BF16, tag="xlabT")
        GRP = 8

        def lab_chunk_dma(eng, g, k):
            eng.dma_start(out=xlabT[:, k, g:g + GRP, :],
                          in_=xlabT_d[k, :, g:g + GRP, :])

        nc.sync.dma_start(out=w1pair[0][:, 0, :], in_=w1p[0][:, 0:H])
        lab_chunk_dma(nc.sync, 0, 0)
        nc.sync.dma_start(out=w1pair[0][:, 1, :], in_=w1p[0][:, H:])
        lab_chunk_dma(nc.sync, 0, 1)
        lab_chunk_dma(nc.sync, 8, 0)
        lab_chunk_dma(nc.sync, 8, 1)
        for g in range(16, N, 2 * GRP):
            for k in range(KL):
                nc.sync.dma_start(
                    out=xlabT[:, k, g:g + 2 * GRP, :],
                    in_=xlabT_d[k, :, g:g + 2 * GRP, :])

        dis_sb = consts.tile([128, N], F32, tag="dis")
        nc.gpsimd.dma_start(out=dis_sb, in_=dis)
        if not zero_b1:
            b1bc = consts.tile([128, H], F32, tag="b1bc")
            b1_bcast = bass.AP(tensor=b1p.tensor, offset=b1p.offset,
                               ap=[[0, 128]] + list(b1p.ap))
            nc.scalar.dma_start(out=b1bc, in_=b1_bcast)

        # --- unlabel branch: unl_h' = xunlT.T @ W1_unl' + b1' (bf16) ---
        unl_sb = consts.tile([128, H], BF16, tag="unl")
        with tc.tile_pool(name="pre_psum", bufs=1, space="PSUM") as pre_psum:
            psu = pre_psum.tile([128, H], F32, tag="psu")
            for half in range(2):
                hs = slice(512 * half, 512 * (half + 1))
                for k in range(KU):
                    nc.tensor.matmul(psu[:, hs], xunlT[k], w1sb[KL + k][:, hs],
                                     start=(k == 0), stop=(k == KU - 1))
            if zero_b1:
                nc.vector.tensor_copy(unl_sb, psu)
            else:
                nc.vector.tensor_tensor(out=unl_sb, in0=psu, in1=b1bc,
                                        op=ALU.add)

        # --- score accumulators (per engine path, pos/neg ranges) ---
        sAp = consts.tile([128, N], F32, tag="sAp")
        sAm = consts.tile([128, N], F32, tag="sAm")
        sDp = consts.tile([128, N], F32, tag="sDp")
        sDm = consts.tile([128, N], F32, tag="sDm")
        for t in (sAp, sAm, sDp, sDm):
            nc.gpsimd.memset(t, 0.0)

        # --- main loop over stations ---
        with tc.tile_pool(name="psum", bufs=4, space="PSUM") as psum_pool, \
             tc.tile_pool(name="relu_sb", bufs=3) as relu_pool:
            # Stations processed in pairs sharing one identity-weight load.
            # For the first pair the inject goes last, so the label matmuls
            # can start before the unlabel chain resolves.
            for n0 in range(0, N, 2):
                pair = (n0, n0 + 1)
                phs = {}
                for n in pair:
                    phs[n] = psum_pool.tile([128, H], F32, tag="ph", name=f"ph_{n}")
                inject_first = n0 >= 16

                def emit_inject(start):
                    for n in pair:
                        for half in range(2):
                            hs = slice(512 * half, 512 * (half + 1))
                            nc.tensor.matmul(phs[n][:, hs], ident,
                                             unl_sb[:, hs],
                                             start=start, stop=not start)

                def emit_lab(first, last):
                    for k in range(KL):
                        for n in pair:
                            for half in range(2):
                                hs = slice(512 * half, 512 * (half + 1))
                                nc.tensor.matmul(
                                    phs[n][:, hs], xlabT[:, k, n, :],
                                    w1sb[k][:, hs],
                                    start=(k == 0 and first),
                                    stop=(k == KL - 1 and last))

                add_dve = ADD_ON_DVE(n0)
                if add_dve:
                    emit_lab(True, True)
                    for n in pair:
                        nc.vector.tensor_tensor(out=phs[n], in0=phs[n],
                                                in1=unl_sb, op=ALU.add)
                elif inject_first:
                    emit_inject(True)
                    emit_lab(False, True)
                else:
                    emit_lab(True, False)
                    emit_inject(False)

                for n in pair:
                    ph = phs[n]
                    _emit_relu(tc, ph, n, jpos, sAp, sAm, sDp, sDm, relu_pool)

        # --- scores assembly + softmax tail (all [128, N] sized) ---
        _emit_tail(tc, consts, out, dis_sb, sAp, sAm, sDp, sDm, b2val)


def _emit_relu(tc, ph, n, jpos, sAp, sAm, sDp, sDm, relu_pool):
    nc = tc.nc
    AF = mybir.ActivationFunctionType
    ALU = mybir.AluOpType
    mode = RELU_MODE(n)
    if mode == 1:
        nc.scalar.activation(
            out=ph[:, :jpos], in_=ph[:, :jpos], func=AF.Relu,
            accum_out=sAp[:, n:n + 1])
        # relu(-x) summed; subtracted at assembly = sum min(x,0)
        nc.scalar.activation(
            out=ph[:, jpos:], in_=ph[:, jpos:], func=AF.Relu,
            scale=-1.0, accum_out=sAm[:, n:n + 1])
    elif mode == 2:
        rl = relu_pool.tile([128, H], BF16, tag="rl")
        nc.scalar.activation(out=rl[:, :jpos], in_=ph[:, :jpos],
                             func=AF.Relu)
        nc.scalar.activation(out=rl[:, jpos:], in_=ph[:, jpos:],
                             func=AF.Relu, scale=-1.0)
        nc.vector.reduce_sum(sDp[:, n:n + 1], rl[:, :jpos],
                             axis=mybir.AxisListType.X)
        nc.vector.reduce_sum(sDm[:, n:n + 1], rl[:, jpos:],
                             axis=mybir.AxisListType.X, negate=True)
    else:
        nc.vector.tensor_scalar(
            out=ph[:, :jpos], in0=ph[:, :jpos], scalar1=0.0,
            scalar2=None, op0=ALU.max, op1=ALU.add,
            accum_out=sDp[:, n:n + 1])
        nc.vector.tensor_scalar(
            out=ph[:, jpos:], in0=ph[:, jpos:], scalar1=0.0,
            scalar2=None, op0=ALU.min, op1=ALU.add,
            accum_out=sDm[:, n:n + 1])


def _emit_tail(tc, consts, out, dis_sb, sAp, sAm, sDp, sDm, b2val):
    # station n used either the A path (sAp - sAm) or the D path
    # (sDp + sDm; sDm holds sum-of-min(x,0) for both modes 2 and 3)
    nc = tc.nc
    AF = mybir.ActivationFunctionType
    ALU = mybir.AluOpType
    t1 = consts.tile([128, N], F32, tag="t1")
    t2 = consts.tile([128, N], F32, tag="t2")
    nc.vector.tensor_tensor(out=t1, in0=sAp, in1=sAm, op=ALU.subtract)
    nc.vector.tensor_tensor(out=t2, in0=sDp, in1=sDm, op=ALU.add)
    nc.vector.tensor_tensor(out=t1, in0=t1, in1=t2, op=ALU.add)
    if b2val != 0.0:
        nc.vector.tensor_scalar_add(t1, t1, float(b2val))
    att = consts.tile([128, N], F32, tag="att")
    nc.vector.tensor_tensor(out=att, in0=t1, in1=dis_sb, op=ALU.mult)

    mxn = consts.tile([128, 1], F32, tag="mxn")
    nc.vector.reduce_max(mxn, att, axis=mybir.AxisListType.X, negate=True)
    ex = consts.tile([128, N], F32, tag="ex")
    sume = consts.tile([128, 1], F32, tag="sume")
    nc.scalar.activation(out=ex, in_=att, func=AF.Exp, bias=mxn,
                         scale=1.0, accum_out=sume)
    rs = consts.tile([128, 1], F32, tag="rs")
    nc.vector.reciprocal(rs, sume)
    res = consts.tile([128, N], F32, tag="res")
    nc.vector.tensor_scalar_mul(res, ex, rs)
    nc.sync.dma_start(out=out, in_=res)


def kernel(unlabel_emb, label_emb, dis_lab, W1, b1, W2, b2):
    global LAST_EXEC_NS, TRACE_DIR
    unlabel_emb = np.asarray(unlabel_emb, dtype=np.float32)
    label_emb = np.asarray(label_emb, dtype=np.float32)
    dis_lab = np.asarray(dis_lab, dtype=np.float32)
    W1 = np.asarray(W1, dtype=np.float32)
    b1 = np.asarray(b1, dtype=np.float32)
    W2 = np.asarray(W2, dtype=np.float32)
    b2 = np.asarray(b2, dtype=np.float32)

    # Fold W2 into W1 columns; sort columns so positive-w2 ones come first.
    w2 = W2[:, 0]
    pos = w2 > 0
    perm = np.argsort(~pos, kind="stable")
    jpos = int(pos.sum())
    W1f = (W1 * w2[None, :])[:, perm]
    b1f = (b1 * w2)[perm]
    b2val = float(b2[0])

    zero_b1 = not np.any(b1f)
    key = (jpos, b2val, zero_b1)
    if key not in _cache:
        _cache[key] = _build(jpos, b2val, zero_b1)
    nc = _cache[key]

    # pair-pack: w1pk[pg, p, k*H + j] = W1f[pg*256 + k*128 + p, j]
    w1p_np = np.ascontiguousarray(
        W1f.reshape(2, 2, 128, H).transpose(0, 2, 1, 3).reshape(2, 128, 2 * H)
    ).astype(ml_dtypes.bfloat16)
    b1p_np = b1f.astype(np.float32)
    in_maps = []
    for c in range(N_CORES):
        sh = slice(c * BS, (c + 1) * BS)
        # [N, BS, EMB] -> [EMB, N, BS] -> [KL, 128, N, BS]
        lab_t = np.ascontiguousarray(
            label_emb[:, sh, :].transpose(2, 0, 1)).reshape(KL, 128, N, BS)
        unl_t = np.ascontiguousarray(
            unlabel_emb[sh].T).reshape(KU, 128, BS)
        in_maps.append({
            "xlabT": lab_t.astype(ml_dtypes.bfloat16),
            "xunlT": unl_t.astype(ml_dtypes.bfloat16),
            "dis": np.ascontiguousarray(dis_lab[sh]),
            "w1p": w1p_np,
            "b1p": b1p_np,
        })

    kwargs = {}
    if PROFILE:
        try:
            import ntff_shim  # noqa: F401  (registers the axon NTFF hook)
        except ImportError:
            pass
        import tempfile
        TRACE_DIR = tempfile.mkdtemp(prefix="bass_trace_")
        kwargs = dict(trace=True, tmpdir=TRACE_DIR)
    res = run_bass_kernel_spmd(nc, in_maps, core_ids=list(range(N_CORES)),
                               **kwargs)
    if PROFILE:
        LAST_EXEC_NS = res.exec_time_ns
    return np.concatenate([res.results[c]["out"] for c in range(N_CORES)],
                          axis=0)


# revision 22
# speedup vs baseline: 1.0160x; 1.0160x over previous
"""Trainium2 Bass kernel for nn_Attention_layer (per-label MLP attention).

Computes, for full inputs:
    h = relu(cat(label_emb, unlabel_emb) @ W1 + b1)        [N, B, H]
    scores = h @ W2 + b2                                   [N, B]
    out = softmax(scores.T * dis_lab, axis=1)              [B, N]

Distribution: pure data-parallel over batch B across 8 NeuronCores
(B=1024 -> 128 rows/core). No collectives; softmax is over the station
axis N which stays local to a core.

Host prep: W2 is folded into W1 (W1' = W1 * w2 per column), columns
sorted so positive-w2 columns come first; then
    scores = sum_pos relu(h') + sum_neg min(h', 0)
Activations/weights are cast to bf16 and label/unlabel embeddings are
pre-transposed to [K, batch] layout so all device DMAs are contiguous.

Device (per core, per station n):
  PE:  psum[128b, 1024] = I @ unl_h' (start) + xlabT_k @ W1'_k   (bf16)
       (the identity matmul injects the shared unlabel contribution -
        an exact rank-128 K-extension - cheaper than any vector add)
  relu+signed-sum per station, balanced across engines, 3 modes:
    1: ACT activation(Relu, accum_out) on [:jpos] / scale=-1 on [jpos:]
    2: ACT relu psum->sbuf bf16, then DVE reduce_sum x2
    3: DVE tensor_scalar(max/min 0, op1=add, accum_out) x2
  Tail: scores * dis_lab, stable softmax over the 64 stations.
"""

import os
import sys

for _p in (
    "/root/.axon_site",
    "/root/.axon_site/_ro/trn_rl_repo",
    "/root/.axon_site/_ro/pypackages",
):
    if _p not in sys.path and os.path.isdir(_p):
        sys.path.append(_p)

import ml_dtypes
import numpy as np

import concourse.bass as bass
import concourse.mybir as mybir
import concourse.tile as tile
from concourse import bacc
from concourse.bass_utils import run_bass_kernel_spmd
from concourse.masks import make_identity

N, B, EMB, UEMB, H = 64, 1024, 256, 256, 1024
N_CORES = 8
BS = B // N_CORES  # 128 batch rows per core
KL = EMB // 128  # label-emb contraction chunks
KU = UEMB // 128  # unlabel-emb contraction chunks
F32 = mybir.dt.float32
BF16 = mybir.dt.bfloat16

# Tuning knobs.
_RELU_MODE_PAT = [1, 2, 2, 3, 1, 2, 2, 3]
RELU_MODE = lambda n: (1 if n == N - 2 else 3) if n >= N - 2 else _RELU_MODE_PAT[n % len(_RELU_MODE_PAT)]  # noqa: E731
ADD_ON_DVE = lambda n0: False  # noqa: E731

PROFILE = False
LAST_EXEC_NS = None
TRACE_DIR = None

_cache = {}


def _build(jpos, b2val, zero_b1):
    nc = bacc.Bacc("TRN2", target_bir_lowering=False, debug=False,
                   num_devices=N_CORES)
    xlabT = nc.dram_tensor("xlabT", [KL, 128, N, BS], BF16,
                           kind="ExternalInput").ap()
    xunlT = nc.dram_tensor("xunlT", [KU, 128, BS], BF16,
                           kind="ExternalInput").ap()
    dis = nc.dram_tensor("dis", [BS, N], F32, kind="ExternalInput").ap()
    w1p = nc.dram_tensor("w1p", [2, 128, 2 * H], BF16,
                         kind="ExternalInput").ap()
    b1p = nc.dram_tensor("b1p", [H], F32, kind="ExternalInput").ap()
    out = nc.dram_tensor("out", [BS, N], F32, kind="ExternalOutput").ap()

    with tile.TileContext(nc) as tc:
        _emit(tc, out, xlabT, xunlT, dis, w1p, b1p, jpos, b2val, zero_b1)
    nc.compile()
    return nc


def _emit(tc, out, xlabT_d, xunlT_d, dis, w1p, b1p, jpos, b2val, zero_b1):
    nc = tc.nc
    AF = mybir.ActivationFunctionType
    ALU = mybir.AluOpType

    with tc.tile_pool(name="consts", bufs=1) as consts:
        # --- constants / weights ---
        ident = consts.tile([128, 128], BF16, tag="ident")
        make_identity(nc, ident)

        w1pair = []
        for pg in range(2):
            t = consts.tile([128, 2, H], BF16, tag=f"w1pg_{pg}")
            w1pair.append(t)
        w1sb = [w1pair[0][:, 0, :], w1pair[0][:, 1, :],
                w1pair[1][:, 0, :], w1pair[1][:, 1, :]]
        # DMA schedule: data arrives in the order the PE consumes it.
        # ACT queue: unl weights (PE-queue head), then tail-only tensors.
        # SP queue: w1[0], first label chunk, w1[1], then remaining chunks.
        # GPSIMD SWDGE: tiny unlabel embeddings (dispatches earliest).
        xunlT = []
        for k in range(KU):
            t = consts.tile([128, 128], BF16, tag=f"xunlT_{k}")
            nc.gpsimd.dma_start(out=t, in_=xunlT_d[k])
            xunlT.append(t)
        nc.scalar.dma_start(out=w1pair[1], in_=w1p[1])

        xlabT = consts.tile([128, KL, N, 128], BF16, tag="xlabT")
        GRP = 8

        def lab_chunk_dma(eng, g, k):
            eng.dma_start(out=xlabT[:, k, g:g + GRP, :],
                          in_=xlabT_d[k, :, g:g + GRP, :])

        nc.sync.dma_start(out=w1pair[0][:, 0, :], in_=w1p[0][:, 0:H])
        for k in range(KL):
            nc.sync.dma_start(out=xlabT[:, k, 0:4, :],
                              in_=xlabT_d[k, :, 0:4, :])
        nc.sync.dma_start(out=w1pair[0][:, 1, :], in_=w1p[0][:, H:])
        for k in range(KL):
            nc.sync.dma_start(out=xlabT[:, k, 4:8, :],
                              in_=xlabT_d[k, :, 4:8, :])
        lab_chunk_dma(nc.sync, 8, 0)
        lab_chunk_dma(nc.sync, 8, 1)
        for g in range(16, N, 2 * GRP):
            for k in range(KL):
                nc.sync.dma_start(
                    out=xlabT[:, k, g:g + 2 * GRP, :],
                    in_=xlabT_d[k, :, g:g + 2 * GRP, :])

        dis_sb = consts.tile([128, N], F32, tag="dis")
        nc.gpsimd.dma_start(out=dis_sb, in_=dis)
        if not zero_b1:
            b1bc = consts.tile([128, H], F32, tag="b1bc")
            b1_bcast = bass.AP(tensor=b1p.tensor, offset=b1p.offset,
                               ap=[[0, 128]] + list(b1p.ap))
            nc.scalar.dma_start(out=b1bc, in_=b1_bcast)

        # --- unlabel branch: unl_h' = xunlT.T @ W1_unl' + b1' (bf16) ---
        unl_sb = consts.tile([128, H], BF16, tag="unl")
        with tc.tile_pool(name="pre_psum", bufs=1, space="PSUM") as pre_psum:
            # PE warmup: dummy matmuls on the identity while input DMAs are
            # still streaming. Keeps the PE busy through the HAM activity
            # window so the real matmul stream starts at the full 2.4 GHz.
            warm = pre_psum.tile([128, 128], F32, tag="warm")
            for w in range(32):
                nc.tensor.matmul(warm, ident, ident,
                                 start=(w == 0), stop=(w == 31))
            psu = pre_psum.tile([128, H], F32, tag="psu")
            for half in range(2):
                hs = slice(512 * half, 512 * (half + 1))
                for k in range(KU):
                    nc.tensor.matmul(psu[:, hs], xunlT[k], w1sb[KL + k][:, hs],
                                     start=(k == 0), stop=(k == KU - 1))
            if zero_b1:
                nc.vector.tensor_copy(unl_sb, psu)
            else:
                nc.vector.tensor_tensor(out=unl_sb, in0=psu, in1=b1bc,
                                        op=ALU.add)

        # --- score accumulators (per engine path, pos/neg ranges) ---
        sAp = consts.tile([128, N], F32, tag="sAp")
        sAm = consts.tile([128, N], F32, tag="sAm")
        sDp = consts.tile([128, N], F32, tag="sDp")
        sDm = consts.tile([128, N], F32, tag="sDm")
        for t in (sAp, sAm, sDp, sDm):
            nc.gpsimd.memset(t, 0.0)

        # --- main loop over stations ---
        with tc.tile_pool(name="psum", bufs=4, space="PSUM") as psum_pool, \
             tc.tile_pool(name="relu_sb", bufs=3) as relu_pool:
            # Stations processed in pairs sharing one identity-weight load.
            # For the first pair the inject goes last, so the label matmuls
            # can start before the unlabel chain resolves.
            for n0 in range(0, N, 2):
                pair = (n0, n0 + 1)
                phs = {}
                for n in pair:
                    phs[n] = psum_pool.tile([128, H], F32, tag="ph", name=f"ph_{n}")
                inject_first = n0 >= 16

                def emit_inject(start):
                    for n in pair:
                        for half in range(2):
                            hs = slice(512 * half, 512 * (half + 1))
                            nc.tensor.matmul(phs[n][:, hs], ident,
                                             unl_sb[:, hs],
                                             start=start, stop=not start)

                def emit_lab(first, last):
                    for k in range(KL):
                        for n in pair:
                            for half in range(2):
                                hs = slice(512 * half, 512 * (half + 1))
                                nc.tensor.matmul(
                                    phs[n][:, hs], xlabT[:, k, n, :],
                                    w1sb[k][:, hs],
                                    start=(k == 0 and first),
                                    stop=(k == KL - 1 and last))

                add_dve = ADD_ON_DVE(n0)
                if add_dve:
                    emit_lab(True, True)
                    for n in pair:
                        nc.vector.tensor_tensor(out=phs[n], in0=phs[n],
                                                in1=unl_sb, op=ALU.add)
                elif inject_first:
                    emit_inject(True)
                    emit_lab(False, True)
                else:
                    emit_lab(True, False)
                    emit_inject(False)

                for n in pair:
                    ph = phs[n]
                    _emit_relu(tc, ph, n, jpos, sAp, sAm, sDp, sDm, relu_pool)

        # --- scores assembly + softmax tail (all [128, N] sized) ---
        _emit_tail(tc, consts, out, dis_sb, sAp, sAm, sDp, sDm, b2val)


def _emit_relu(tc, ph, n, jpos, sAp, sAm, sDp, sDm, relu_pool):
    nc = tc.nc
    AF = mybir.ActivationFunctionType
    ALU = mybir.AluOpType
    mode = RELU_MODE(n)
    if mode == 1:
        nc.scalar.activation(
            out=ph[:, :jpos], in_=ph[:, :jpos], func=AF.Relu,
            accum_out=sAp[:, n:n + 1])
        # relu(-x) summed; subtracted at assembly = sum min(x,0)
        nc.scalar.activation(
            out=ph[:, jpos:], in_=ph[:, jpos:], func=AF.Relu,
            scale=-1.0, accum_out=sAm[:, n:n + 1])
    elif mode == 2:
        rl = relu_pool.tile([128, H], BF16, tag="rl")
        nc.scalar.activation(out=rl[:, :jpos], in_=ph[:, :jpos],
                             func=AF.Relu)
        nc.scalar.activation(out=rl[:, jpos:], in_=ph[:, jpos:],
                             func=AF.Relu, scale=-1.0)
        nc.vector.reduce_sum(sDp[:, n:n + 1], rl[:, :jpos],
                             axis=mybir.AxisListType.X)
        nc.vector.reduce_sum(sDm[:, n:n + 1], rl[:, jpos:],
                             axis=mybir.AxisListType.X, negate=True)
    else:
        nc.vector.tensor_scalar(
            out=ph[:, :jpos], in0=ph[:, :jpos], scalar1=0.0,
            scalar2=None, op0=ALU.max, op1=ALU.add,
            accum_out=sDp[:, n:n + 1])
        nc.vector.tensor_scalar(
            out=ph[:, jpos:], in0=ph[:, jpos:], scalar1=0.0,
            scalar2=None, op0=ALU.min, op1=ALU.add,
            accum_out=sDm[:, n:n + 1])


def _emit_tail(tc, consts, out, dis_sb, sAp, sAm, sDp, sDm, b2val):
    # station n used either the A path (sAp - sAm) or the D path
    # (sDp + sDm; sDm holds sum-of-min(x,0) for both modes 2 and 3)
    nc = tc.nc
    AF = mybir.ActivationFunctionType
    ALU = mybir.AluOpType
    t1 = consts.tile([128, N], F32, tag="t1")
    t2 = consts.tile([128, N], F32, tag="t2")
    nc.vector.tensor_tensor(out=t1, in0=sAp, in1=sAm, op=ALU.subtract)
    nc.vector.tensor_tensor(out=t2, in0=sDp, in1=sDm, op=ALU.add)
    nc.vector.tensor_tensor(out=t1, in0=t1, in1=t2, op=ALU.add)
    if b2val != 0.0:
        nc.vector.tensor_scalar_add(t1, t1, float(b2val))
    att = consts.tile([128, N], F32, tag="att")
    nc.vector.tensor_tensor(out=att, in0=t1, in1=dis_sb, op=ALU.mult)

    mxn = consts.tile([128, 1], F32, tag="mxn")
    nc.vector.reduce_max(mxn, att, axis=mybir.AxisListType.X, negate=True)
    ex = consts.tile([128, N], F32, tag="ex")
    sume = consts.tile([128, 1], F32, tag="sume")
    nc.scalar.activation(out=ex, in_=att, func=AF.Exp, bias=mxn,
                         scale=1.0, accum_out=sume)
    rs = consts.tile([128, 1], F32, tag="rs")
    nc.vector.reciprocal(rs, sume)
    res = consts.tile([128, N], F32, tag="res")
    nc.vector.tensor_scalar_mul(res, ex, rs)
    nc.sync.dma_start(out=out[:64, :], in_=res[:64, :])
    nc.scalar.dma_start(out=out[64:, :], in_=res[64:, :])


def kernel(unlabel_emb, label_emb, dis_lab, W1, b1, W2, b2):
    global LAST_EXEC_NS, TRACE_DIR
    unlabel_emb = np.asarray(unlabel_emb, dtype=np.float32)
    label_emb = np.asarray(label_emb, dtype=np.float32)
    dis_lab = np.asarray(dis_lab, dtype=np.float32)
    W1 = np.asarray(W1, dtype=np.float32)
    b1 = np.asarray(b1, dtype=np.float32)
    W2 = np.asarray(W2, dtype=np.float32)
    b2 = np.asarray(b2, dtype=np.float32)

    # Fold W2 into W1 columns; sort columns so positive-w2 ones come first.
    w2 = W2[:, 0]
    pos = w2 > 0
    perm = np.argsort(~pos, kind="stable")
    jpos = int(pos.sum())
    W1f = (W1 * w2[None, :])[:, perm]
    b1f = (b1 * w2)[perm]
    b2val = float(b2[0])

    zero_b1 = not np.any(b1f)
    key = (jpos, b2val, zero_b1)
    if key not in _cache:
        _cache[key] = _build(jpos, b2val, zero_b1)
    nc = _cache[key]

    # pair-pack: w1pk[pg, p, k*H + j] = W1f[pg*256 + k*128 + p, j]
    w1p_np = np.ascontiguousarray(
        W1f.reshape(2, 2, 128, H).transpose(0, 2, 1, 3).reshape(2, 128, 2 * H)
    ).astype(ml_dtypes.bfloat16)
    b1p_np = b1f.astype(np.float32)
    in_maps = []
    for c in range(N_CORES):
        sh = slice(c * BS, (c + 1) * BS)
        # [N, BS, EMB] -> [EMB, N, BS] -> [KL, 128, N, BS]
        lab_t = np.ascontiguousarray(
            label_emb[:, sh, :].transpose(2, 0, 1)).reshape(KL, 128, N, BS)
        unl_t = np.ascontiguousarray(
            unlabel_emb[sh].T).reshape(KU, 128, BS)
        in_maps.append({
            "xlabT": lab_t.astype(ml_dtypes.bfloat16),
            "xunlT": unl_t.astype(ml_dtypes.bfloat16),
            "dis": np.ascontiguousarray(dis_lab[sh]),
            "w1p": w1p_np,
            "b1p": b1p_np,
        })

    kwargs = {}
    if PROFILE:
        try:
            import ntff_shim  # noqa: F401  (registers the axon NTFF hook)
        except ImportError:
            pass
        import tempfile
        TRACE_DIR = tempfile.mkdtemp(prefix="bass_trace_")
        kwargs = dict(trace=True, tmpdir=TRACE_DIR)
    res = run_bass_kernel_spmd(nc, in_maps, core_ids=list(range(N_CORES)),
                               **kwargs)
    if PROFILE:
        LAST_EXEC_NS = res.exec_time_ns
    return np.concatenate([res.results[c]["out"] for c in range(N_CORES)],
                          axis=0)
